# revision 2
# baseline (speedup 1.0000x reference)
"""Trainium2 Bass kernel for nn_AttentiveBPNet (grouped attention scoring).

Math (exact algebraic reduction of the reference):
  sk = x @ wk, sv = x @ wv (wk/wv [C,H] folded from W_att/att on host).
  Per group g: score[a,b,t,h] = lrelu(sk[ik(g,a,t),h] + sv[iv(g,b,t),h]),
  mean over t, softmax over b (M=2 -> sigmoid of difference).

Distribution / algorithm (8 cores, data-parallel over G):
  - Each core owns 1024 groups, processed as 4 quarter-batches of 256.
  - Host prep per (core, qb): dedup/sort the <=16384 unique node ids its
    16384 slots reference; ship x[U]^T as a [128, 16384] bf16 tile
    (channels on partitions) plus int16 compact slot indices.
  - Device per qb: TensorE projects the unique nodes into an SBUF score
    table [128, u, 2] whose partition q%8 holds head h=q%8 and whose
    d-axis interleaves (sk_h, sv_h); ap_gather (8 gpsimd cores, one per
    16-partition band = 32 groups) resolves the per-slot lookups fully
    inside SBUF; DVE forms lrelu pairs via lrelu(z)=0.6z+0.4|z| and
    reduces over t; ACT applies the sigmoid softmax.
  - No SWDGE descriptors anywhere on the critical path: slot resolution
    runs on the 8 Q7 cores in parallel at ~27ns/slot/core.
"""

import os

import numpy as np
import ml_dtypes

import concourse.bacc as bacc
import concourse.bass as bass
import concourse.tile as tile
from concourse import mybir, bass_utils, library_config

NCORES = 8
N, C, H, M, S, G = 200000, 64, 8, 2, 16, 8192
SLOPE = 0.2
GPC = G // NCORES            # 1024 groups per core
QB = 4                       # quarter-batches per core
GPQ = GPC // QB              # 256 groups per quarter-batch
GBAND = GPQ // 8             # 32 groups per gpsimd band
NSLOT = GPQ * M * S          # 8192 slots per (qb, list)
BSLOT = NSLOT // 8           # 1024 slots per band per list
UCAP = 2 * NSLOT             # 16384-row compact table
CHUNK = 512                  # matmul free-dim chunk
F32 = mybir.dt.float32
BF16 = mybir.dt.bfloat16
I16 = mybir.dt.int16

_cache: dict = {}


def _build_nc():
    nc = bacc.Bacc(trn_type="TRN2", num_devices=NCORES)
    xtT = nc.declare_dram_parameter("xtT", [QB, 128, UCAP], BF16, isOutput=False)
    wt = nc.declare_dram_parameter("wt", [64, 256], BF16, isOutput=False)
    ixs = nc.declare_dram_parameter("ixs", [QB, 128, 2, BSLOT // 16], I16,
                                    isOutput=False)
    yout = nc.declare_dram_parameter("yout", [QB, 128, GBAND * M * M], F32,
                                     isOutput=True)

    with tile.TileContext(nc) as tc:
        with (
            tc.tile_pool(name="const", bufs=1) as cpool,
            tc.tile_pool(name="xin", bufs=1) as xpool,
            tc.tile_pool(name="tab", bufs=2) as tpool,
            tc.tile_pool(name="idx", bufs=2) as ipool,
            tc.tile_pool(name="psum", bufs=4, space="PSUM") as ppool,
            tc.tile_pool(name="gath", bufs=2) as gpool,
            tc.tile_pool(name="z", bufs=2) as zpool,
            tc.tile_pool(name="small", bufs=4) as mpool,
        ):
            nc.gpsimd.load_library(library_config.ap_gather)
            w_sb = cpool.tile([64, 256], BF16)
            nc.sync.dma_start(w_sb[:, :], wt[:, :])

            for qb in range(QB):
                xt_sb = xpool.tile([128, UCAP], BF16, tag="xt")
                nc.sync.dma_start(xt_sb[:, :], xtT[qb, :, :])
                ix_sb = ipool.tile([128, 2 * (BSLOT // 16)], I16, tag="ix")
                nc.sync.dma_start(
                    ix_sb[:, :].rearrange("p (l n) -> p l n", l=2),
                    ixs[qb, :, :, :],
                )
                table = tpool.tile([128, UCAP * 2], BF16, tag="tab")
                tv = table[:, :].rearrange("p (n s) -> p n s", s=2)
                for ck in range(UCAP // CHUNK):
                    lo = ck * CHUNK
                    psA = ppool.tile([128, CHUNK], F32, tag="psA")
                    psB = ppool.tile([128, CHUNK], F32, tag="psB")
                    nc.tensor.matmul(
                        psA[:, :],
                        lhsT=w_sb[:, 0:128],
                        rhs=xt_sb[0:64, lo : lo + CHUNK],
                        start=True,
                        stop=True,
                    )
                    nc.tensor.matmul(
                        psB[:, :],
                        lhsT=w_sb[:, 128:256],
                        rhs=xt_sb[0:64, lo : lo + CHUNK],
                        start=True,
                        stop=True,
                    )
                    nc.vector.tensor_copy(tv[:, lo : lo + CHUNK, 0], psA[:, :])
                    nc.scalar.activation(
                        out=tv[:, lo : lo + CHUNK, 1],
                        in_=psB[:, :],
                        func=mybir.ActivationFunctionType.Copy,
                        scale=1.0,
                    )
                outs = []
                for kv in range(2):
                    ot = gpool.tile([128, BSLOT * 2], BF16, tag=f"o{kv}")
                    nc.gpsimd.ap_gather(
                        ot[:, :].rearrange("p (n s) -> p n s", s=2),
                        tv,
                        ix_sb[:, kv * (BSLOT // 16) : (kv + 1) * (BSLOT // 16)],
                        128,
                        UCAP,
                        2,
                        BSLOT,
                    )
                    outs.append(ot)
                ok, ov_ = outs
                # z[p, g, a, b, t] = K[p, (g,a,t), 0] + V[p, (g,b,t), 1]
                okv = ok[:, :].rearrange(
                    "p (g a t s) -> p g a t s", g=GBAND, a=M, t=S, s=2
                )
                ovv = ov_[:, :].rearrange(
                    "p (g b t s) -> p g b t s", g=GBAND, b=M, t=S, s=2
                )
                z = zpool.tile([128, GBAND * M * M * S], F32, tag="z")
                zv = z[:, :].rearrange(
                    "p (g a b t) -> p g a b t", g=GBAND, a=M, b=M, t=S
                )
                for a in range(M):
                    for b in range(M):
                        nc.vector.tensor_tensor(
                            out=zv[:, :, a, b, :],
                            in0=okv[:, :, a, :, 0],
                            in1=ovv[:, :, b, :, 1],
                            op=mybir.AluOpType.add,
                        )
                zr = z[:, :].rearrange(
                    "p (q t) -> p q t", q=GBAND * M * M, t=S
                )
                s_abs = mpool.tile([128, GBAND * M * M], F32, tag="sabs")
                nc.vector.tensor_reduce(
                    out=s_abs[:, :],
                    in_=zr,
                    axis=mybir.AxisListType.X,
                    op=mybir.AluOpType.add,
                    apply_absolute_value=True,
                )
                s_z = mpool.tile([128, GBAND * M * M], F32, tag="sz")
                nc.vector.tensor_reduce(
                    out=s_z[:, :],
                    in_=zr,
                    axis=mybir.AxisListType.X,
                    op=mybir.AluOpType.add,
                )
                t2 = mpool.tile([128, GBAND * M * M], F32, tag="t2")
                nc.vector.tensor_scalar(
                    out=t2[:, :],
                    in0=s_z[:, :],
                    scalar1=1.5,
                    scalar2=None,
                    op0=mybir.AluOpType.mult,
                )
                nc.vector.tensor_tensor(
                    out=t2[:, :],
                    in0=t2[:, :],
                    in1=s_abs[:, :],
                    op=mybir.AluOpType.add,
                )
                t2v = t2[:, :].rearrange(
                    "p (g a b) -> p g a b", g=GBAND, a=M, b=M
                )
                d = mpool.tile([128, GBAND * M], F32, tag="d")
                dv = d[:, :].rearrange("p (g a) -> p g a", g=GBAND, a=M)
                nc.vector.tensor_tensor(
                    out=dv,
                    in0=t2v[:, :, :, 0],
                    in1=t2v[:, :, :, 1],
                    op=mybir.AluOpType.subtract,
                )
                out_t = mpool.tile([128, GBAND * M * M], F32, tag="out")
                ovt = out_t[:, :].rearrange(
                    "p (g a b) -> p g a b", g=GBAND, a=M, b=M
                )
                nc.scalar.activation(
                    out=ovt[:, :, :, 0],
                    in_=dv,
                    func=mybir.ActivationFunctionType.Sigmoid,
                    scale=SLOPE * 2.0 / ((M * S) // 2),
                )
                nc.vector.tensor_scalar(
                    out=ovt[:, :, :, 1],
                    in0=ovt[:, :, :, 0],
                    scalar1=-1.0,
                    scalar2=1.0,
                    op0=mybir.AluOpType.mult,
                    op1=mybir.AluOpType.add,
                )
                nc.sync.dma_start(yout[qb, :, :], out_t[:, :])
    nc.finalize()
    return nc


def _fold_w2(W_att, att):
    Wr = W_att.reshape(C, H, C)
    wk = np.einsum("dhc,hc->dh", Wr, att[:, :C])
    wv = np.einsum("dhc,hc->dh", Wr, att[:, C:])
    return wk.astype(np.float32), wv.astype(np.float32)


def _wrap16(pos):
    """[BSLOT] -> [16, BSLOT//16] wrapped (idx i at [i%16, i//16])."""
    return pos.reshape(BSLOT // 16, 16).T.astype(np.int16)


def prepare_inputs(x, node_idxes, W_att, att):
    x = np.asarray(x, dtype=np.float32)
    W_att = np.asarray(W_att, dtype=np.float32)
    att = np.asarray(att, dtype=np.float32)
    ni = np.asarray(node_idxes)

    wk, wv = _fold_w2(W_att, att)
    wt = np.concatenate(
        [np.tile(wk, (1, 16)), np.tile(wv, (1, 16))], axis=1
    ).astype(ml_dtypes.bfloat16)  # [64, 256]

    x_bf = x.astype(ml_dtypes.bfloat16)

    idx_k = ni[:, :, 1, :]  # [G, M, S] key list (index a)
    idx_v = ni[:, :, 0, :]  # [G, M, S] value list (index b)

    in_maps = []
    for c in range(NCORES):
        xtT = np.zeros((QB, 128, UCAP), dtype=ml_dtypes.bfloat16)
        ixs = np.empty((QB, 128, 2, BSLOT // 16), dtype=np.int16)
        for qb in range(QB):
            g0 = c * GPC + qb * GPQ
            kf = idx_k[g0 : g0 + GPQ].reshape(-1)  # [NSLOT] (g,a,t)
            vf = idx_v[g0 : g0 + GPQ].reshape(-1)
            u = np.unique(np.concatenate([kf, vf]))
            assert len(u) <= UCAP
            xtT[qb, 0:64, : len(u)] = x_bf[u].T
            kp = np.searchsorted(u, kf)
            vp = np.searchsorted(u, vf)
            for band in range(8):
                sl = slice(band * BSLOT, (band + 1) * BSLOT)
                ixs[qb, 16 * band : 16 * band + 16, 0] = _wrap16(kp[sl])
                ixs[qb, 16 * band : 16 * band + 16, 1] = _wrap16(vp[sl])
        in_maps.append({"xtT": xtT, "wt": wt, "ixs": ixs})
    return in_maps


def kernel(x, edge_index, node_idxes, W_att, att, **_unused):
    in_maps = prepare_inputs(x, node_idxes, W_att, att)
    if "nc" not in _cache:
        _cache["nc"] = _build_nc()
    nc = _cache["nc"]

    trace = bool(int(os.environ.get("KERNEL_TRACE", "0")))
    res = bass_utils.run_bass_kernel_spmd(
        nc, in_maps, core_ids=list(range(NCORES)), trace=trace
    )
    _cache["last_result"] = res
    out = np.empty((G, M, M, H), dtype=np.float32)
    for c in range(NCORES):
        y = res.results[c]["yout"]  # [QB, 128, GBAND*M*M]
        # partition p = 16*band + q; q in [0,8): h = q (rep 0)
        y = y.reshape(QB, 8, 2, H, GBAND, M, M)[:, :, 0]
        y = y.transpose(0, 1, 3, 4, 5, 2)  # [qb, band, g32, a, b, h]
        out[c * GPC : (c + 1) * GPC] = y.reshape(GPC, M, M, H)
    return out
